# revision 1
# baseline (speedup 1.0000x reference)
"""Trainium2 Bass kernel for the masked-FFT CG data-consistency problem.

Math: the reference runs 10 CG iterations on (A^H A + lam I) x = atbT + lam z
where A^H A = ifft2(mask * fft2(.)) is DIAGONAL in the Fourier basis with
eigenvalue d = mask + lam per mode.  CG therefore collapses: with per-mode
weights w_j = sum_b |rhs_hat[b, j]|^2, every CG scalar is an integral against
(d, w), so the whole iteration reduces to a scalar recurrence producing one
filter map chi(d_j), and  out = ifft2(chi * fft2(rhs)).

Device work = batched 512x512 FFT2 / IFFT2, run as radix-2 DFT matmuls
(float32r, 1 cycle/row) on the tensor engine, batch-sharded 2 slices/core
across 8 cores.  Kernel A computes rhs_hat and partial w; the tiny collapsed
CG runs on host; kernel B applies chi and inverse-transforms.

FFT2 per slice is two matmul passes with the DATA blocks stationary and the
DFT matrices moving: pass(X) = (F @ X).T, so pass(pass(X)) = F X F = fft2(X)
with no transposes.  Radix-2: rows split even/odd (K=256 each), twiddles
folded into the odd-part moving matrix, E +/- T recombined on the vector
engine during PSUM eviction.  Rows live in a parity-grouped layout
sigma(jt, p) = 2*((jt % 2)*128 + p) + jt//2, preserved across passes by
selecting stride-2 column blocks, so no partition permutes are ever needed.
"""

import numpy as np

LAM = 0.05
CG_ITER = 10
B, H, W = 16, 512, 512
JT, P = 4, 128
N_CORES = 8

_cache = {}


def _perm_rows():
    idx = np.zeros(512, np.int64)
    for jt in range(4):
        for p in range(128):
            idx[jt * 128 + p] = 2 * ((jt % 2) * 128 + p) + jt // 2
    return idx


def _make_consts2():
    m = np.arange(256)
    k1 = np.arange(256)
    we = np.exp(-2j * np.pi * np.outer(m, k1) / 256)
    wt = we * np.exp(-2j * np.pi * k1 / 512)[None, :]

    def arr(M):
        return np.ascontiguousarray(M.astype(np.float32).reshape(2, 128, 256))

    return (arr(we.real), arr(we.imag), arr(-we.imag),
            arr(wt.real), arr(wt.imag), arr(-wt.imag))


def _collapsed_cg(d, w, iters=CG_ITER, tol=1e-10):
    d = d.astype(np.float64).ravel()
    w = w.astype(np.float64).ravel()
    q = np.ones_like(d)
    s = np.ones_like(d)
    chi = np.zeros_like(d)
    rTr = (q * q * w).sum()
    for _ in range(iters):
        if abs(rTr) <= tol:
            break
        denom = (d * s * s * w).sum()
        alpha = rTr / denom
        chi = chi + alpha * s
        q = q - alpha * d * s
        rTr_new = (q * q * w).sum()
        beta = rTr_new / rTr
        s = q + beta * s
        rTr = rTr_new
    return chi.reshape(512, 512)


def _build_kernels():
    import concourse.mybir as mybir
    import concourse.tile as tile
    from concourse import bacc

    dt_mm = mybir.dt.float32r

    def load_consts(nc, cpool, aps):
        names = ["ger", "gei", "gein", "gtr", "gti", "gtin"]
        tiles = []
        for name, ap in zip(names, aps):
            t = cpool.tile([P, 2, 256], dt_mm, tag=name)
            nc.sync.dma_start(t[:], ap.rearrange("kt p c -> p kt c"))
            tiles.append(t)
        return tiles

    def dft_pass(nc, psp, dpool, plane_r, plane_i, G, emit, conj=False):
        ger, gei, gein, gtr, gti, gtin = G
        for q in range(4):
            start = 256 * (q % 2) + q // 2
            ps_r = psp.tile([P, W], mybir.dt.float32, tag="psr")
            ps_i = psp.tile([P, W], mybir.dt.float32, tag="psi")
            for part, jts, gr, gi, gin in (
                ("E", (0, 1), ger, gei, gein),
                ("T", (2, 3), gtr, gti, gtin),
            ):
                off = 0 if part == "E" else 256
                orr = ps_r[:, off:off + 256]
                oii = ps_i[:, off:off + 256]
                for kt in range(2):
                    xr = plane_r[:, jts[kt], start:start + 255:2]
                    xi = plane_i[:, jts[kt], start:start + 255:2]
                    first, last = kt == 0, kt == 1
                    if not conj:
                        nc.tensor.matmul(orr, xr, gr[:, kt, :], start=first, stop=False)
                        nc.tensor.matmul(orr, xi, gin[:, kt, :], start=False, stop=last)
                        nc.tensor.matmul(oii, xr, gi[:, kt, :], start=first, stop=False)
                        nc.tensor.matmul(oii, xi, gr[:, kt, :], start=False, stop=last)
                    else:
                        nc.tensor.matmul(orr, xr, gr[:, kt, :], start=first, stop=False)
                        nc.tensor.matmul(orr, xi, gi[:, kt, :], start=False, stop=last)
                        nc.tensor.matmul(oii, xi, gr[:, kt, :], start=first, stop=False)
                        nc.tensor.matmul(oii, xr, gin[:, kt, :], start=False, stop=last)
            emit(q, ps_r, ps_i)

    def combine(nc, dpool, dst_lo, dst_hi, ps, tag):
        tsb = dpool.tile([P, 256], mybir.dt.float32, tag=tag)
        nc.scalar.copy(tsb[:], ps[:, 256:512])
        nc.vector.tensor_add(dst_lo, ps[:, 0:256], tsb[:])
        nc.vector.tensor_sub(dst_hi, ps[:, 0:256], tsb[:])

    def build_a():
        nc = bacc.Bacc("TRN2", target_bir_lowering=False, debug=False,
                       num_devices=N_CORES)
        zs = nc.dram_tensor("zs", [2, H, W, 2], mybir.dt.float32, kind="ExternalInput").ap()
        as_ = nc.dram_tensor("as_", [2, H, W, 2], mybir.dt.float32, kind="ExternalInput").ap()
        gaps = [nc.dram_tensor(n, [2, P, 256], dt_mm, kind="ExternalInput").ap()
                for n in ["ger", "gei", "gein", "gtr", "gti", "gtin"]]
        hh = nc.dram_tensor("hh", [2, 2, JT, P, W], mybir.dt.float32, kind="ExternalOutput").ap()
        wo = nc.dram_tensor("wo", [JT, P, W], mybir.dt.float32, kind="ExternalOutput").ap()

        with tile.TileContext(nc) as tc:
            with (
                tc.tile_pool(name="const", bufs=1) as cpool,
                tc.tile_pool(name="data", bufs=2) as dpool,
                tc.tile_pool(name="ps", bufs=4, space="PSUM") as psp,
            ):
                G = load_consts(nc, cpool, gaps)
                wacc = cpool.tile([P, JT, W], mybir.dt.float32, tag="w")
                nc.vector.memset(wacc[:], 0.0)

                for b in range(2):
                    zt = dpool.tile([P, JT, W, 2], mybir.dt.float32, tag="z")
                    at = dpool.tile([P, JT, W, 2], mybir.dt.float32, tag="a")
                    src = "b (sub p par) c k -> b p par sub c k"
                    v = "p (par sub) c k -> p par sub c k"
                    nc.sync.dma_start(zt[:].rearrange(v, par=2, sub=2),
                                      zs.rearrange(src, sub=2, p=P, par=2)[b])
                    nc.sync.dma_start(at[:].rearrange(v, par=2, sub=2),
                                      as_.rearrange(src, sub=2, p=P, par=2)[b])
                    rt = dpool.tile([P, JT, W, 2], dt_mm, tag="r")
                    nc.scalar.mul(zt[:], zt[:], LAM)
                    nc.gpsimd.tensor_add(rt[:], at[:], zt[:])

                    ar = dpool.tile([P, JT, W], dt_mm, tag="ar")
                    ai = dpool.tile([P, JT, W], dt_mm, tag="ai")

                    def emit_a(q, ps_r, ps_i):
                        combine(nc, dpool, ar[:, q, 0:256], ar[:, q, 256:512], ps_r, "tr")
                        combine(nc, dpool, ai[:, q, 0:256], ai[:, q, 256:512], ps_i, "ti")

                    dft_pass(nc, psp, dpool, rt[:, :, :, 0], rt[:, :, :, 1], G, emit_a)

                    hr = dpool.tile([P, JT, W], mybir.dt.float32, tag="hr")
                    hi = dpool.tile([P, JT, W], mybir.dt.float32, tag="hi")

                    def emit_h(q, ps_r, ps_i):
                        combine(nc, dpool, hr[:, q, 0:256], hr[:, q, 256:512], ps_r, "tr")
                        combine(nc, dpool, hi[:, q, 0:256], hi[:, q, 256:512], ps_i, "ti")
                        sq = dpool.tile([P, W], mybir.dt.float32, tag="sq")
                        nc.scalar.square(sq[:], hr[:, q, :])
                        nc.gpsimd.tensor_add(wacc[:, q, :], wacc[:, q, :], sq[:])
                        sq2 = dpool.tile([P, W], mybir.dt.float32, tag="sq2")
                        nc.scalar.square(sq2[:], hi[:, q, :])
                        nc.gpsimd.tensor_add(wacc[:, q, :], wacc[:, q, :], sq2[:])

                    dft_pass(nc, psp, dpool, ar[:], ai[:], G, emit_h)

                    nc.sync.dma_start(hh.rearrange("b k jt p c -> b k p jt c")[b, 0], hr[:])
                    nc.sync.dma_start(hh.rearrange("b k jt p c -> b k p jt c")[b, 1], hi[:])

                nc.sync.dma_start(wo.rearrange("jt p c -> p jt c"), wacc[:])

        nc.compile()
        return nc

    def build_b():
        nc = bacc.Bacc("TRN2", target_bir_lowering=False, debug=False,
                       num_devices=N_CORES)
        hh = nc.dram_tensor("hh", [2, 2, JT, P, W], mybir.dt.float32, kind="ExternalInput").ap()
        chi = nc.dram_tensor("chi", [JT, P, W], mybir.dt.float32, kind="ExternalInput").ap()
        gaps = [nc.dram_tensor(n, [2, P, 256], dt_mm, kind="ExternalInput").ap()
                for n in ["ger", "gei", "gein", "gtr", "gti", "gtin"]]
        out = nc.dram_tensor("out", [2, H, W, 2], mybir.dt.float32, kind="ExternalOutput").ap()

        with tile.TileContext(nc) as tc:
            with (
                tc.tile_pool(name="const", bufs=1) as cpool,
                tc.tile_pool(name="data", bufs=2) as dpool,
                tc.tile_pool(name="ps", bufs=4, space="PSUM") as psp,
            ):
                G = load_consts(nc, cpool, gaps)
                cht = cpool.tile([P, JT, W], mybir.dt.float32, tag="chi")
                nc.sync.dma_start(cht[:], chi.rearrange("jt p c -> p jt c"))

                for b in range(2):
                    gr = dpool.tile([P, JT, W], dt_mm, tag="gr")
                    gi = dpool.tile([P, JT, W], dt_mm, tag="gi")
                    hrt = dpool.tile([P, JT, W], mybir.dt.float32, tag="hrt")
                    hit = dpool.tile([P, JT, W], mybir.dt.float32, tag="hit")
                    nc.sync.dma_start(hrt[:], hh.rearrange("b k jt p c -> b k p jt c")[b, 0])
                    nc.sync.dma_start(hit[:], hh.rearrange("b k jt p c -> b k p jt c")[b, 1])
                    nc.vector.tensor_mul(gr[:], hrt[:], cht[:])
                    nc.gpsimd.tensor_mul(gi[:], hit[:], cht[:])

                    ar = dpool.tile([P, JT, W], dt_mm, tag="ar")
                    ai = dpool.tile([P, JT, W], dt_mm, tag="ai")

                    def emit_a(q, ps_r, ps_i):
                        combine(nc, dpool, ar[:, q, 0:256], ar[:, q, 256:512], ps_r, "tr")
                        combine(nc, dpool, ai[:, q, 0:256], ai[:, q, 256:512], ps_i, "ti")

                    dft_pass(nc, psp, dpool, gr[:], gi[:], G, emit_a, conj=True)

                    oi = dpool.tile([P, JT, W, 2], mybir.dt.float32, tag="oi")

                    def emit_o(q, ps_r, ps_i):
                        combine(nc, dpool, oi[:, q, 0:256, 0], oi[:, q, 256:512, 0], ps_r, "tr")
                        combine(nc, dpool, oi[:, q, 0:256, 1], oi[:, q, 256:512, 1], ps_i, "ti")

                    dft_pass(nc, psp, dpool, ar[:], ai[:], G, emit_o, conj=True)

                    dstp = "b (sub p par) c k -> b p par sub c k"
                    nc.sync.dma_start(
                        out.rearrange(dstp, sub=2, p=P, par=2)[b],
                        oi[:].rearrange("p (par sub) c k -> p par sub c k", par=2, sub=2))

        nc.compile()
        return nc

    return build_a(), build_b()


LAST_EXEC_NS = {}


def kernel(z, atbT, mask):
    import os
    from concourse.bass_utils import run_bass_kernel_spmd

    trace = bool(os.environ.get("DC_TRACE"))

    if "k" not in _cache:
        _cache["k"] = _build_kernels()
    nca, ncb = _cache["k"]

    G = dict(zip(["ger", "gei", "gein", "gtr", "gti", "gtin"], _make_consts2()))
    perm = _perm_rows()

    z = np.ascontiguousarray(np.asarray(z, dtype=np.float32))
    atbT = np.ascontiguousarray(np.asarray(atbT, dtype=np.float32))
    mask = np.asarray(mask, dtype=np.float32)

    in_a = [
        {"zs": np.ascontiguousarray(z[2 * c:2 * c + 2]),
         "as_": np.ascontiguousarray(atbT[2 * c:2 * c + 2]), **G}
        for c in range(N_CORES)
    ]
    res_a = run_bass_kernel_spmd(nca, in_a, core_ids=list(range(N_CORES)), trace=trace)
    if trace:
        LAST_EXEC_NS["a"] = res_a.exec_time_ns

    w_total = np.zeros((JT, P, W), np.float64)
    for c in range(N_CORES):
        w_total += res_a.results[c]["wo"].astype(np.float64)
    d_dev = (mask.astype(np.float64) + LAM)[perm]
    chi_dev = (_collapsed_cg(d_dev, w_total.reshape(512, 512)) / (512.0 * 512.0))
    chi_t = np.ascontiguousarray(chi_dev.astype(np.float32).reshape(JT, P, W))

    in_b = [{"hh": res_a.results[c]["hh"], "chi": chi_t, **G} for c in range(N_CORES)]
    res_b = run_bass_kernel_spmd(ncb, in_b, core_ids=list(range(N_CORES)), trace=trace)
    if trace:
        LAST_EXEC_NS["b"] = res_b.exec_time_ns

    return np.concatenate([res_b.results[c]["out"] for c in range(N_CORES)], axis=0)


# revision 4
# speedup vs baseline: 1.0023x; 1.0023x over previous
"""Trainium2 Bass kernel for the masked-FFT CG data-consistency problem.

Math: the reference runs 10 CG iterations on (A^H A + lam I) x = atbT + lam z
where A^H A = ifft2(mask * fft2(.)) is DIAGONAL in the Fourier basis with
eigenvalue d = mask + lam per mode.  CG therefore collapses: with per-mode
weights w_j = sum_b |rhs_hat[b, j]|^2 every CG scalar is an integral against
(d, w), so the 10 iterations reduce to a tiny scalar recurrence producing one
filter map chi(d_j), and  out = ifft2(chi * fft2(rhs)).

Device work = batched 512x512 FFT2 / IFFT2 as radix-2 DFT matmuls (float32r,
1 cycle/row on the PE) batch-sharded 2 slices/core over 8 cores.
Kernel A: rhs = atbT + lam z; rhs_hat = FFT2(rhs); partial w.  Host: the
collapsed CG (numpy, ~1 ms).  Kernel B: chi * rhs_hat; IFFT2; emit output.

Each FFT2 is two matmul passes with the DATA blocks stationary and the DFT
matrices moving: pass(X) = (F @ X).T, so pass(pass(X)) = F X F = fft2(X), no
transposes.  Radix-2 splits rows even/odd (K=256 per part, twiddles folded
into the odd-part moving matrices); moving consts pack [re|im] halves so one
matmul fills [E_re|E_im] of a PSUM bank; E +/- T recombines on the vector
engine during eviction (T staged through SBUF by the scalar engine - DVE
cannot read two PSUM operands).  Rows use a parity-grouped layout
sigma(jt, p) = 2*((jt % 2)*128 + p) + jt//2, preserved across passes by
selecting stride-2 column blocks, so no partition permutes are needed.
bf16 dummy matmuls warm the PE HAM clock while input DMAs stream.
"""

import numpy as np

LAM = 0.05
CG_ITER = 10
B_FULL, H, W = 16, 512, 512
JT, P = 4, 128
N_CORES = 8

_cache = {}


def _perm_rows():
    idx = np.zeros(512, np.int64)
    for jt in range(4):
        for p in range(128):
            idx[jt * 128 + p] = 2 * ((jt % 2) * 128 + p) + jt // 2
    return idx


def _make_consts(conj):
    m = np.arange(256)
    k1 = np.arange(256)
    we = np.exp(-2j * np.pi * np.outer(m, k1) / 256)
    wt = we * np.exp(-2j * np.pi * k1 / 512)[None, :]

    def comp(a, b):
        M = np.concatenate([a, b], axis=1)
        return np.ascontiguousarray(M.astype(np.float32).reshape(2, 128, 512))

    if not conj:
        return (comp(we.real, we.imag), comp(-we.imag, we.real),
                comp(wt.real, wt.imag), comp(-wt.imag, wt.real))
    return (comp(we.real, -we.imag), comp(we.imag, we.real),
            comp(wt.real, -wt.imag), comp(wt.imag, wt.real))


def _collapsed_cg(d, w, iters=CG_ITER, tol=1e-10):
    d = d.astype(np.float64).ravel()
    w = w.astype(np.float64).ravel()
    q = np.ones_like(d)
    s = np.ones_like(d)
    chi = np.zeros_like(d)
    rTr = (q * q * w).sum()
    for _ in range(iters):
        if abs(rTr) <= tol:
            break
        denom = (d * s * s * w).sum()
        alpha = rTr / denom
        chi = chi + alpha * s
        q = q - alpha * d * s
        rTr_new = (q * q * w).sum()
        beta = rTr_new / rTr
        s = q + beta * s
        rTr = rTr_new
    return chi.reshape(512, 512)


def _build_kernels():
    import concourse.mybir as mybir
    import concourse.tile as tile
    from concourse import bacc

    dt_mm = mybir.dt.float32r

    def load_consts(nc, cpool, aps):
        tiles = []
        for name, ap in zip(["a1", "a2", "t1", "t2"], aps):
            t = cpool.tile([P, 2, 512], dt_mm, tag=name)
            nc.sync.dma_start(t[:], ap.rearrange("kt p c -> p kt c"))
            tiles.append(t)
        return tiles

    def warmup(nc, cpool, psp, n=28):
        wb = cpool.tile([P, 128], mybir.dt.bfloat16, tag="wb")
        mb = cpool.tile([P, 512], mybir.dt.bfloat16, tag="mb")
        nc.vector.memset(wb[:], 0.0)
        nc.vector.memset(mb[:], 0.0)
        for _ in range(n):
            pw = psp.tile([P, 512], mybir.dt.float32, tag="pse")
            nc.tensor.matmul(pw[:], wb[:], mb[:], start=True, stop=True)

    def dft_pass(nc, psp, dpool, stat, G3, emit, qs=(0, 1, 2, 3)):
        a1, a2, t1, t2 = G3
        for q in qs:
            ps_e = psp.tile([P, 512], mybir.dt.float32, tag="pse")
            ps_t = psp.tile([P, 512], mybir.dt.float32, tag="pst")
            for part, jts, m1, m2 in (("E", (0, 1), a1, a2), ("T", (2, 3), t1, t2)):
                ps = ps_e if part == "E" else ps_t
                for kt in range(2):
                    nc.tensor.matmul(ps[:], stat(jts[kt], q, 0), m1[:, kt, :],
                                     start=(kt == 0), stop=False)
                    nc.tensor.matmul(ps[:], stat(jts[kt], q, 1), m2[:, kt, :],
                                     start=False, stop=(kt == 1))
            t_sb = dpool.tile([P, 512], mybir.dt.float32, tag="tsb")
            nc.scalar.copy(t_sb[:], ps_t[:])
            emit(q, ps_e, t_sb)

    def comb(nc, plane, q, ps_e, t_sb):
        e2 = ps_e[:].rearrange("p (k c) -> p k c", k=2)
        t2 = t_sb[:].rearrange("p (k c) -> p k c", k=2)
        nc.vector.tensor_add(plane[:, q, :, 0:256], e2, t2)
        nc.vector.tensor_sub(plane[:, q, :, 256:512], e2, t2)

    def build_a():
        nc = bacc.Bacc("TRN2", target_bir_lowering=False, debug=False,
                       num_devices=N_CORES)
        zs = nc.dram_tensor("zs", [2, H, W, 2], mybir.dt.float32, kind="ExternalInput").ap()
        as_ = nc.dram_tensor("as_", [2, H, W, 2], mybir.dt.float32, kind="ExternalInput").ap()
        gaps = [nc.dram_tensor(n, [2, P, 512], dt_mm, kind="ExternalInput").ap()
                for n in ["a1", "a2", "t1", "t2"]]
        hh = nc.dram_tensor("hh", [2, JT, 2, P, W], mybir.dt.float32, kind="ExternalOutput").ap()
        wo = nc.dram_tensor("wo", [JT, P, W], mybir.dt.float32, kind="ExternalOutput").ap()

        with tile.TileContext(nc) as tc:
            with (
                tc.tile_pool(name="const", bufs=1) as cpool,
                tc.tile_pool(name="data", bufs=2) as dpool,
                tc.tile_pool(name="ps", bufs=3, space="PSUM") as psp,
            ):
                src = "b (sub p par) c k -> b p par sub c k"
                v = "p (par sub) c k -> p par sub c k"
                zts, ats, rts = [], [], []
                for b in range(2):
                    zt = dpool.tile([P, JT, W, 2], mybir.dt.float32, tag="z")
                    at = dpool.tile([P, JT, W, 2], mybir.dt.float32, tag="a")
                    rt = dpool.tile([P, JT, W, 2], dt_mm, tag="r")
                    zts.append(zt)
                    ats.append(at)
                    rts.append(rt)
                G3 = None
                for b, cc in ((0, 0), (0, 1), (1, 0), (1, 1)):
                    cs = slice(cc * 256, (cc + 1) * 256)
                    zv = zts[b][:].rearrange(v, par=2, sub=2)
                    av = ats[b][:].rearrange(v, par=2, sub=2)
                    nc.sync.dma_start(
                        zv[:, :, :, cs, :],
                        zs.rearrange(src, sub=2, p=P, par=2)[b][:, :, :, cs, :])
                    nc.sync.dma_start(
                        av[:, :, :, cs, :],
                        as_.rearrange(src, sub=2, p=P, par=2)[b][:, :, :, cs, :])
                    if b == 0 and cc == 0:
                        G3 = load_consts(nc, cpool, gaps)
                warmup(nc, cpool, psp)
                wacc = cpool.tile([P, JT, W], mybir.dt.float32, tag="w")
                nc.vector.memset(wacc[:], 0.0)

                for b in range(2):
                    zt, at, rt = zts[b], ats[b], rts[b]
                    for cc in range(2):
                        cs = slice(cc * 256, (cc + 1) * 256)
                        nc.scalar.mul(zt[:, :, cs, :], zt[:, :, cs, :], LAM)
                        nc.vector.tensor_add(rt[:, :, cs, :], at[:, :, cs, :],
                                             zt[:, :, cs, :])

                    ar = dpool.tile([P, JT, 2, W], dt_mm, tag="ar")

                    def stat1(jt, q, comp, rt=rt):
                        start = 256 * (q % 2) + q // 2
                        return rt[:, jt, start:start + 255:2, comp]

                    def emit_a(q, ps_e, t_sb, ar=ar):
                        comb(nc, ar, q, ps_e, t_sb)

                    dft_pass(nc, psp, dpool, stat1, G3, emit_a, qs=(0, 2, 1, 3))

                    hr = dpool.tile([P, JT, 2, W], mybir.dt.float32, tag="hr")

                    def stat2(jt, q, comp, ar=ar):
                        start = 256 * (q % 2) + q // 2
                        return ar[:, jt, comp, start:start + 255:2]

                    def emit_h(q, ps_e, t_sb, b=b, hr=hr):
                        comb(nc, hr, q, ps_e, t_sb)
                        sq = dpool.tile([P, 2, W], mybir.dt.float32, tag="sq")
                        nc.scalar.square(sq[:], hr[:, q, :, :])
                        nc.gpsimd.tensor_add(wacc[:, q, :], wacc[:, q, :], sq[:, 0, :])
                        nc.gpsimd.tensor_add(wacc[:, q, :], wacc[:, q, :], sq[:, 1, :])
                        nc.sync.dma_start(
                            hh.rearrange("b q k p c -> b p q k c")[b][:, q], hr[:, q])
                        if b == 1:
                            nc.sync.dma_start(
                                wo.rearrange("jt p c -> p jt c")[:, q], wacc[:, q, :])

                    dft_pass(nc, psp, dpool, stat2, G3, emit_h)

        nc.compile()
        return nc

    def build_b():
        nc = bacc.Bacc("TRN2", target_bir_lowering=False, debug=False,
                       num_devices=N_CORES)
        hh = nc.dram_tensor("hh", [2, JT, 2, P, W], mybir.dt.float32, kind="ExternalInput").ap()
        chi = nc.dram_tensor("chi", [JT, P, W], mybir.dt.float32, kind="ExternalInput").ap()
        gaps = [nc.dram_tensor(n, [2, P, 512], dt_mm, kind="ExternalInput").ap()
                for n in ["a1", "a2", "t1", "t2"]]
        out = nc.dram_tensor("out", [2, H, W, 2], mybir.dt.float32, kind="ExternalOutput").ap()

        with tile.TileContext(nc) as tc:
            with (
                tc.tile_pool(name="const", bufs=1) as cpool,
                tc.tile_pool(name="data", bufs=2) as dpool,
                tc.tile_pool(name="ps", bufs=3, space="PSUM") as psp,
            ):
                cht = cpool.tile([P, JT, W], mybir.dt.float32, tag="chi")
                hts, gts = [], []
                for b in range(2):
                    ht = dpool.tile([P, JT, 2, W], mybir.dt.float32, tag="ht")
                    gt = dpool.tile([P, JT, 2, W], dt_mm, tag="gt")
                    hts.append(ht)
                    gts.append(gt)
                hv = hh.rearrange("b q k p c -> b p q k c")
                chv = chi.rearrange("jt p c -> p jt c")
                nc.sync.dma_start(hts[0][:, 0], hv[0][:, 0])
                nc.sync.dma_start(cht[:, 0, :], chv[:, 0, :])
                G3 = load_consts(nc, cpool, gaps)
                for q in range(1, 4):
                    nc.sync.dma_start(cht[:, q, :], chv[:, q, :])
                for b in range(2):
                    for q in range(4):
                        if not (b == 0 and q == 0):
                            nc.sync.dma_start(hts[b][:, q], hv[b][:, q])
                warmup(nc, cpool, psp, n=40)

                for b in range(2):
                    ht, gt = hts[b], gts[b]
                    for q in range(4):
                        nc.vector.tensor_mul(gt[:, q, 0, :], ht[:, q, 0, :], cht[:, q, :])
                        nc.gpsimd.tensor_mul(gt[:, q, 1, :], ht[:, q, 1, :], cht[:, q, :])

                    ar = dpool.tile([P, JT, 2, W], dt_mm, tag="ar")

                    def stat1(jt, q, comp, gt=gt):
                        start = 256 * (q % 2) + q // 2
                        return gt[:, jt, comp, start:start + 255:2]

                    def emit_a(q, ps_e, t_sb, ar=ar):
                        comb(nc, ar, q, ps_e, t_sb)

                    dft_pass(nc, psp, dpool, stat1, G3, emit_a)

                    oi = dpool.tile([P, JT, W, 2], mybir.dt.float32, tag="oi")

                    def stat2(jt, q, comp, ar=ar):
                        start = 256 * (q % 2) + q // 2
                        return ar[:, jt, comp, start:start + 255:2]

                    def emit_o(q, ps_e, t_sb, b=b, oi=oi):
                        e2 = ps_e[:].rearrange("p (k c) -> p k c", k=2)
                        t2 = t_sb[:].rearrange("p (k c) -> p k c", k=2)
                        lo = oi[:, q, 0:256, :].rearrange("p c k -> p k c")
                        hi = oi[:, q, 256:512, :].rearrange("p c k -> p k c")
                        nc.vector.tensor_add(lo, e2, t2)
                        nc.vector.tensor_sub(hi, e2, t2)
                        dstp = "b (sub p par) c k -> b p par sub c k"
                        ov = out.rearrange(dstp, sub=2, p=P, par=2)[b]
                        nc.sync.dma_start(ov[:, q // 2, q % 2], oi[:, q])

                    dft_pass(nc, psp, dpool, stat2, G3, emit_o)

        nc.compile()
        return nc

    return build_a(), build_b()


LAST_EXEC_NS = {}


def kernel(z, atbT, mask):
    import os
    from concourse.bass_utils import run_bass_kernel_spmd

    trace = bool(os.environ.get("DC_TRACE"))

    if "k" not in _cache:
        _cache["k"] = _build_kernels()
    nca, ncb = _cache["k"]

    Gf = dict(zip(["a1", "a2", "t1", "t2"], _make_consts(conj=False)))
    Gc = dict(zip(["a1", "a2", "t1", "t2"], _make_consts(conj=True)))
    perm = _perm_rows()

    z = np.ascontiguousarray(np.asarray(z, dtype=np.float32))
    atbT = np.ascontiguousarray(np.asarray(atbT, dtype=np.float32))
    mask = np.asarray(mask, dtype=np.float32)

    in_a = [
        {"zs": np.ascontiguousarray(z[2 * c:2 * c + 2]),
         "as_": np.ascontiguousarray(atbT[2 * c:2 * c + 2]), **Gf}
        for c in range(N_CORES)
    ]
    res_a = run_bass_kernel_spmd(nca, in_a, core_ids=list(range(N_CORES)), trace=trace)
    if trace:
        LAST_EXEC_NS["a"] = res_a.exec_time_ns

    w_total = np.zeros((JT, P, W), np.float64)
    for c in range(N_CORES):
        w_total += res_a.results[c]["wo"].astype(np.float64)
    d_dev = (mask.astype(np.float64) + LAM)[perm]
    chi_dev = _collapsed_cg(d_dev, w_total.reshape(512, 512)) / (512.0 * 512.0)
    chi_t = np.ascontiguousarray(chi_dev.astype(np.float32).reshape(JT, P, W))

    in_b = [{"hh": res_a.results[c]["hh"], "chi": chi_t, **Gc} for c in range(N_CORES)]
    res_b = run_bass_kernel_spmd(ncb, in_b, core_ids=list(range(N_CORES)), trace=trace)
    if trace:
        LAST_EXEC_NS["b"] = res_b.exec_time_ns

    return np.concatenate([res_b.results[c]["out"] for c in range(N_CORES)], axis=0)
